# revision 49
# baseline (speedup 1.0000x reference)
"""BinaryConv (XNOR-style binary-weight 3x3 conv) on 8 Trainium2 NeuronCores.

Full-input contract: kernel(x=[32,256,56,56] f32, weight=[256,256,3,3] f32)
-> [32,256,56,56] f32.

Strategy: data-parallel over batch (4 images/core), weight replicated.
Per core, an implicit GEMM over the 9 conv taps in fp8-e4m3 DoubleRow
(double-pumped) matmuls:
  out[co, hw] = a[co] * ( sum_t  s_t^T x8  +  sum_{t in T5} s_t^T r8 )
where x8 = e4m3(x) and r8 = e4m3(x - x8) are a two-term fp8 expansion of
x (host-marshalled dtype split), s = sign(w) is exact in fp8 and ships
pre-packed in the stationary layout, and the residual correction over 5
of the 9 taps brings the e4m3 quantization error (2.65e-2 single-pass)
down to 1.78e-2 rel — under the 2e-2 gate — while costing (9+5)/18 =
0.78x of the bf16 PE work at the 2x fp8 rate.

Each DoubleRow matmul contracts all 256 input channels at once (both
128-channel chunks via the [p, 2, ...] k-tile layout) over an 8-row
output block (N=448 <= 512 fp32 psum bank). Taps are emitted
stationary-outer: for each (image, co-chunk) the 7 row-block psum tiles
accumulate tap-by-tap, so the 5 residual taps start ~12us after the PE
does, hiding the r8 staging DMA + interior copy on the ramp; the final
(image, co) block flips to rowblock-outer so its evictions pipeline with
the last matmuls instead of bunching after them.

The fp32 scale a[co]=mean|w[co]| is computed on device from a bf16 copy
of the weight (1e-4 rel to the f32 mean) and applied at PSUM eviction in
fp32, alternating DVE tensor_scalar and ACT activation-with-scale by row
block. PE warmup matmuls hold the HAM clock at 2.4GHz through the
DMA-bound ramp; input DMAs are latency-ordered on the sync HWDGE ring;
interior copies split DVE/ACT; borders are gpsimd memsets; output stores
ride the scalar ring. Prefetch DMAs for image n+1 are deferred to the
middle of image n so they don't compete with the ramp-critical loads.
"""

import ml_dtypes
import numpy as np

import concourse.mybir as mybir
import concourse.tile as tile
from concourse import bacc
from concourse.bass_utils import run_bass_kernel_spmd

F32 = mybir.dt.float32
BF16 = mybir.dt.bfloat16
F8 = mybir.dt.float8e4

N_CORES = 8
B, C, H, W = 32, 256, 56, 56
O, KH, KW = 256, 3, 3
BP = B // N_CORES            # images per core
PH, PW = H + 2, W + 2        # padded spatial
P = 128                      # partitions
NCI = C // P                 # input-channel chunks (k-tiles per matmul)
NCO = O // P                 # output-channel chunks
HT = 8                       # output rows per psum tile
NFREE = HT * W               # 448 <= 512 fp32 psum bank
NHT = H // HT                # 7
NTAP = KH * KW               # 9
NTC = 5                      # residual-corrected taps (taps 0..4)
KIN = C * NTAP               # 2304 = per-filter fan-in
WTF = NTAP * NCO * NCI * P   # 4608 = packed lhsT free size


def _wt_off(t: int, co: int) -> int:
    # packed stationary layout: [co-chunk][tap][k-tile][oo] — co-major so
    # the first co block's ramp only waits on half the weight bytes
    return (co * NTAP + t) * NCI * P


def build(bp: int = BP):
    """Build + compile the per-core program for `bp` images per core."""
    nc = bacc.Bacc(
        "TRN2",
        target_bir_lowering=False,
        debug=False,
        enable_asserts=False,
        num_devices=N_CORES,
        enable_partition_id=False,
    )
    x8_d = nc.dram_tensor("x8", [bp, C, H * W], F8, kind="ExternalInput")
    r8_d = nc.dram_tensor("r8", [bp, C, H * W], F8, kind="ExternalInput")
    wb_d = nc.dram_tensor("wb", [O, KIN], BF16, kind="ExternalInput")
    # wp8[p, t, co, i, oo] = sign(w[co*128+oo, i*128+p, t]) in fp8 — the
    # packed DoubleRow stationary, host-marshalled.
    wp8_d = nc.dram_tensor("wp8", [P, WTF], F8, kind="ExternalInput")
    out_d = nc.dram_tensor("out", [bp, O, H, W], F32, kind="ExternalOutput")

    x8 = x8_d.ap().rearrange("n (c p) v -> p n c v", p=P)
    r8 = r8_d.ap().rearrange("n (c p) v -> p n c v", p=P)
    out = out_d.ap().rearrange("n c h w -> n c (h w)")

    COPY = mybir.ActivationFunctionType.Copy

    with tile.TileContext(nc) as tc:
        with (
            tc.tile_pool(name="const", bufs=1) as const_pool,
            tc.tile_pool(name="wstage", bufs=2) as wstage_pool,
            tc.tile_pool(name="xsf", bufs=4) as xsf_pool,
            tc.tile_pool(name="xpad", bufs=4) as xpad_pool,
            tc.tile_pool(name="otile", bufs=8) as out_pool,
            tc.tile_pool(name="psum", bufs=7, space="PSUM") as psum_pool,
            tc.tile_pool(name="warmps", bufs=1, space="PSUM") as warmps_pool,
        ):
            # ---- ramp-critical x8 chunk-1 DMAs: very first gpsimd ops ----
            # (the gpsimd ring can issue DMAs; enqueueing before its
            # memset backlog keeps the critical transfer start at ~engine
            # init instead of ~3.5us later). Image 0's staging is split
            # into 14-row pieces so the first matmul waits on 0.2MB, not
            # 0.8MB — the first-block group schedule below consumes
            # pieces as they land.
            HH = H // 2
            NPC = 4                  # image-0 row pieces
            PCR = H // NPC           # 14 rows per piece
            x8s0 = [[xsf_pool.tile([P, PCR * W], F8, name=f"x8s{k}_{ci}",
                                   bufs=1) for ci in range(NCI)]
                    for k in range(NPC)]
            # gpsimd's engine-driven transfers run at full rate even
            # before the clock boost — it carries ALL image-0 pieces in
            # consumption order (~5.3us chain, done before group 2 needs
            # them); the HWDGE rings are too slow pre-boost and would
            # starve the middle groups
            for k in range(NPC):
                for ci in range(NCI):
                    nc.gpsimd.dma_start(
                        x8s0[k][ci][:],
                        x8[:, 0, ci, k * PCR * W:(k + 1) * PCR * W])

            # ---- PE warmup: keep HAM at 2.4GHz while inputs stream in ----
            # warm operands memset on DVE: the gpsimd queue is busy with
            # the staging transfers above and would delay the warmups
            warm_l = const_pool.tile([P, P], BF16)
            warm_r = const_pool.tile([P, 512], BF16)
            nc.vector.memset(warm_l[:], 0.0)
            nc.vector.memset(warm_r[:], 0.0)
            zbias = const_pool.tile([P, 1], F32)
            zscr = const_pool.tile([P, 1], F32)
            nc.vector.memset(zbias[:], 0.0)
            warm_ps = warmps_pool.tile([P, 512], F32)
            # enough back-to-back zero matmuls to keep the PE busy from
            # engine-start until the first real matmul: the HAM 2.4GHz
            # boost engages after ~4us of SUSTAINED Tensor activity, and
            # an idle gap before the first real matmul re-arms the timer.
            # The ramp-critical ~1.4MB can't land before ~12us (early
            # DMA runs pre-boost too), so the warmups bridge that window
            # and real matmuls start already at full clock.
            N_WARM = 16  # covers data-ready jitter of ~12.5-16us
            for _ in range(N_WARM):
                nc.tensor.matmul(warm_ps[:], warm_l[:], warm_r[:],
                                 start=True, stop=True)
            # preload the Copy LUT on ACT before evictions need it
            nc.scalar.copy(zscr[:], zbias[:])

            wt8 = const_pool.tile([P, WTF], F8)
            a_all = const_pool.tile([P, NCO], F32)

            def pad_alloc():
                xp = xpad_pool.tile([P, NCI, PH, PW], F8, name="xp")
                nc.gpsimd.memset(xp[:, :, 0, :], 0.0)
                nc.gpsimd.memset(xp[:, :, PH - 1, :], 0.0)
                nc.gpsimd.memset(xp[:, :, 1:PH - 1, 0], 0.0)
                nc.gpsimd.memset(xp[:, :, 1:PH - 1, PW - 1], 0.0)
                return xp

            # ---- ramp-critical input DMAs --------------------------------
            # Only x8s0 + the first weight taps transfer up front; r8s0/wb
            # are enqueued on the scalar ring BEHIND an ACT copy that
            # blocks on x8s0's arrival (engine rings execute in order, so
            # the transfers physically start after the critical one).
            # The image-0 staging is split top/bottom so the interior
            # copies pipeline with the second half of the DMA.
            xp0 = pad_alloc()
            rp0 = pad_alloc()
            r8s0 = [xsf_pool.tile([P, NCI, HH * W], F8, name=f"r8s{h}",
                                  bufs=1) for h in range(2)]
            # chunk-0 pieces on the sync ring (chunk-1 rode gpsimd above);
            # the first weight taps share the window, the rest follows
            WCO = WTF // 2
            nc.scalar.dma_start(wt8[:, :WCO], wp8_d.ap()[:, :WCO])
            nc.sync.dma_start(wt8[:, WCO:], wp8_d.ap()[:, WCO:])
            wstages = [wstage_pool.tile([P, KIN], BF16, name="ws")
                       for _ in range(NCO)]
            wb = wb_d.ap()

            # ---- first-image interior copies: piece-pipelined DVE / ACT --
            for k in range(NPC):
                nc.vector.tensor_copy(
                    xp0[:, 0, 1 + k * PCR:1 + (k + 1) * PCR, 1:PW - 1],
                    x8s0[k][0][:].rearrange("p (h v) -> p h v", v=W))
                nc.scalar.copy(
                    xp0[:, 1, 1 + k * PCR:1 + (k + 1) * PCR, 1:PW - 1],
                    x8s0[k][1][:].rearrange("p (h v) -> p h v", v=W))
            # r8s0/wb start here — after all of x8s0+wt8 has landed, and
            # well before the corrective taps need them (~10us later)
            nc.scalar.dma_start(r8s0[0][:], r8[:, 0, :, :HH * W])
            nc.scalar.dma_start(r8s0[1][:], r8[:, 0, :, HH * W:])
            nc.scalar.dma_start(wstages[0][:], wb[0:P, :])
            nc.scalar.dma_start(wstages[1][:], wb[P:2 * P, :])
            for half in range(2):
                nc.vector.tensor_copy(
                    rp0[:, 0, 1 + half * HH:1 + (half + 1) * HH, 1:PW - 1],
                    r8s0[half][:, 0].rearrange("p (h v) -> p h v", v=W))
                nc.scalar.copy(
                    rp0[:, 1, 1 + half * HH:1 + (half + 1) * HH, 1:PW - 1],
                    r8s0[half][:, 1].rearrange("p (h v) -> p h v", v=W))

            # |w| means (DVE) — per co chunk, before that chunk's evictions
            def emit_reduce(c2):
                asum = wstage_pool.tile([P, 1], F32, name="asum", bufs=2)
                nc.vector.tensor_reduce(
                    asum[:], wstages[c2][:], axis=mybir.AxisListType.X,
                    op=mybir.AluOpType.add, apply_absolute_value=True,
                )
                nc.vector.tensor_scalar_mul(
                    a_all[:, c2:c2 + 1], asum[:], 1.0 / KIN
                )

            def lhs(t, co):
                off = _wt_off(t, co)
                return wt8[:, off:off + NCI * P].rearrange(
                    "p (i o) -> p i o", i=NCI)

            def rhs(xp, t, ht):
                kh, kw = divmod(t, KW)
                r0 = ht * HT + kh
                return xp[:, :, r0:r0 + HT, kw:kw + W]

            def evict(ps, n, co, ht, split=False):
                ot = out_pool.tile([P, NFREE], F32, name="ot")
                if ht % 2 == 0:
                    nc.vector.tensor_scalar_mul(
                        ot[:], ps[:], a_all[:, co:co + 1])
                else:
                    nc.scalar.activation(
                        ot[:], ps[:], COPY, bias=0.0,
                        scale=a_all[:, co:co + 1])
                dst = out[n, co * P:(co + 1) * P,
                          ht * NFREE:(ht + 1) * NFREE]
                if split:
                    # final-block stores ride the idle gpsimd engine
                    # (DIRECT2D, ~1.5us per tile) instead of waiting
                    # ~4.8us for the HWDGE queue descriptor drain
                    nc.gpsimd.dma_start(dst, ot[:])
                else:
                    nc.scalar.dma_start(dst, ot[:])

            DR = mybir.MatmulPerfMode.DoubleRow

            # ---- main conv loop ------------------------------------------
            xp, rp = xp0, rp0
            for n in range(bp):
                x8s = r8s = None
                for co in range(NCO):
                    if n == 0:
                        emit_reduce(co)
                    final = (n == bp - 1 and co == NCO - 1)
                    pss = [psum_pool.tile([P, NFREE], F32, name="ps")
                           for _ in range(NHT)]
                    if not final:
                        # the very first block consumes staging pieces as
                        # they land: each group's row blocks fit within
                        # the pieces available at that point, keeping the
                        # PE fed without a clock-dropping idle gap
                        if n == 0 and co == 0:
                            ht_groups = [[0], [1, 2], [3, 4], [5, 6]]
                        else:
                            ht_groups = [range(NHT)]
                        for hts in ht_groups:
                            for t in range(NTAP):
                                lt = lhs(t, co)
                                for ht in hts:
                                    nc.tensor.matmul(
                                        pss[ht][:], lt, rhs(xp, t, ht),
                                        start=(t == 0), stop=False,
                                        perf_mode=DR)
                        for hts in ht_groups:
                            for t in range(NTC):
                                lt = lhs(t, co)
                                for ht in hts:
                                    nc.tensor.matmul(
                                        pss[ht][:], lt, rhs(rp, t, ht),
                                        start=False, stop=(t == NTC - 1),
                                        perf_mode=DR)
                                    if t == NTC - 1:
                                        evict(pss[ht], n, co, ht)
                    else:
                        # rowblock-outer: pipeline evictions with matmuls
                        for ht in range(NHT):
                            for t in range(NTAP):
                                nc.tensor.matmul(
                                    pss[ht][:], lhs(t, co), rhs(xp, t, ht),
                                    start=(t == 0), stop=False,
                                    perf_mode=DR)
                            for t in range(NTC):
                                nc.tensor.matmul(
                                    pss[ht][:], lhs(t, co), rhs(rp, t, ht),
                                    start=False, stop=(t == NTC - 1),
                                    perf_mode=DR)
                            evict(pss[ht], n, co, ht, split=True)
                    # next image's prefetch DMAs ride the scalar ring
                    # behind co-0's evictions: the transfers start only
                    # once the ramp/previous traffic has drained, without
                    # stealing HBM bandwidth from the critical loads (the
                    # sync engine would run ahead and start them at t=0).
                    # The interior copies are emitted between co0 and co1
                    # so DVE/ACT reach them during co1's compute, a full
                    # co-block before the next image's matmuls need them.
                    if co == 0 and n + 1 < bp:
                        x8s = xsf_pool.tile([P, NCI, H * W], F8,
                                            name="x8f", bufs=2)
                        r8s = xsf_pool.tile([P, NCI, H * W], F8,
                                            name="r8f", bufs=2)
                        nc.scalar.dma_start(x8s[:], x8[:, n + 1])
                        nc.scalar.dma_start(r8s[:], r8[:, n + 1])
                        nxp = pad_alloc()
                        nrp = pad_alloc()
                        nc.vector.tensor_copy(
                            nxp[:, 0, 1:PH - 1, 1:PW - 1],
                            x8s[:, 0].rearrange("p (h v) -> p h v", v=W))
                        nc.scalar.copy(
                            nxp[:, 1, 1:PH - 1, 1:PW - 1],
                            x8s[:, 1].rearrange("p (h v) -> p h v", v=W))
                        nc.vector.tensor_copy(
                            nrp[:, 0, 1:PH - 1, 1:PW - 1],
                            r8s[:, 0].rearrange("p (h v) -> p h v", v=W))
                        nc.scalar.copy(
                            nrp[:, 1, 1:PH - 1, 1:PW - 1],
                            r8s[:, 1].rearrange("p (h v) -> p h v", v=W))
                if n + 1 < bp:
                    xp, rp = nxp, nrp

    nc.compile()
    return nc


_NC_CACHE: dict[int, object] = {}


def _get_nc(bp: int = BP):
    if bp not in _NC_CACHE:
        _NC_CACHE[bp] = build(bp)
    return _NC_CACHE[bp]


def make_in_maps(x: np.ndarray, weight: np.ndarray, n_cores: int = N_CORES,
                 bp: int = BP):
    x = np.ascontiguousarray(x, dtype=np.float32).reshape(B, C, H * W)
    weight = np.ascontiguousarray(weight, dtype=np.float32)
    x8 = x.astype(ml_dtypes.float8_e4m3)
    r8 = (x - x8.astype(np.float32)).astype(ml_dtypes.float8_e4m3)
    # wp8[p, t, co, i, oo] = sign(w[co*128+oo, i*128+p, t]) — exact in fp8
    wv = np.sign(weight).reshape(NCO, P, NCI, P, NTAP)  # [co, oo, i, p, t]
    wp8 = np.ascontiguousarray(
        wv.transpose(3, 0, 4, 2, 1)                     # [p, co, t, i, oo]
    ).reshape(P, WTF).astype(ml_dtypes.float8_e4m3)
    wb = weight.reshape(O, KIN).astype(ml_dtypes.bfloat16)
    return [
        {"x8": x8[i * bp:(i + 1) * bp], "r8": r8[i * bp:(i + 1) * bp],
         "wb": wb, "wp8": wp8}
        for i in range(n_cores)
    ]


def kernel(x: np.ndarray, weight: np.ndarray) -> np.ndarray:
    nc = _get_nc(BP)
    in_maps = make_in_maps(x, weight)
    res = run_bass_kernel_spmd(nc, in_maps, core_ids=list(range(N_CORES)))
    out = np.empty((B, O, H, W), dtype=np.float32)
    for i in range(N_CORES):
        out[i * BP:(i + 1) * BP] = res.results[i]["out"].reshape(BP, O, H, W)
    return out


# revision 50
# speedup vs baseline: 1.0122x; 1.0122x over previous
"""BinaryConv (XNOR-style binary-weight 3x3 conv) on 8 Trainium2 NeuronCores.

Full-input contract: kernel(x=[32,256,56,56] f32, weight=[256,256,3,3] f32)
-> [32,256,56,56] f32.

Strategy: data-parallel over batch (4 images/core), weight replicated.
Per core, an implicit GEMM over the 9 conv taps in fp8-e4m3 DoubleRow
(double-pumped) matmuls:
  out[co, hw] = a[co] * ( sum_t  s_t^T x8  +  sum_{t in T5} s_t^T r8 )
where x8 = e4m3(x) and r8 = e4m3(x - x8) are a two-term fp8 expansion of
x (host-marshalled dtype split), s = sign(w) is exact in fp8 and ships
pre-packed in the stationary layout, and the residual correction over 5
of the 9 taps brings the e4m3 quantization error (2.65e-2 single-pass)
down to 1.78e-2 rel — under the 2e-2 gate — while costing (9+5)/18 =
0.78x of the bf16 PE work at the 2x fp8 rate.

Each DoubleRow matmul contracts all 256 input channels at once (both
128-channel chunks via the [p, 2, ...] k-tile layout) over an 8-row
output block (N=448 <= 512 fp32 psum bank). Taps are emitted
stationary-outer: for each (image, co-chunk) the 7 row-block psum tiles
accumulate tap-by-tap, so the 5 residual taps start ~12us after the PE
does, hiding the r8 staging DMA + interior copy on the ramp; the final
(image, co) block flips to rowblock-outer so its evictions pipeline with
the last matmuls instead of bunching after them.

The fp32 scale a[co]=mean|w[co]| is computed on device from a bf16 copy
of the weight (1e-4 rel to the f32 mean) and applied at PSUM eviction in
fp32, alternating DVE tensor_scalar and ACT activation-with-scale by row
block. PE warmup matmuls hold the HAM clock at 2.4GHz through the
DMA-bound ramp; input DMAs are latency-ordered on the sync HWDGE ring;
interior copies split DVE/ACT; borders are gpsimd memsets; output stores
ride the scalar ring. Prefetch DMAs for image n+1 are deferred to the
middle of image n so they don't compete with the ramp-critical loads.
"""

import ml_dtypes
import numpy as np

import concourse.mybir as mybir
import concourse.tile as tile
from concourse import bacc
from concourse.bass_utils import run_bass_kernel_spmd

F32 = mybir.dt.float32
BF16 = mybir.dt.bfloat16
F8 = mybir.dt.float8e4

N_CORES = 8
B, C, H, W = 32, 256, 56, 56
O, KH, KW = 256, 3, 3
BP = B // N_CORES            # images per core
PH, PW = H + 2, W + 2        # padded spatial
P = 128                      # partitions
NCI = C // P                 # input-channel chunks (k-tiles per matmul)
NCO = O // P                 # output-channel chunks
HT = 8                       # output rows per psum tile
NFREE = HT * W               # 448 <= 512 fp32 psum bank
NHT = H // HT                # 7
NTAP = KH * KW               # 9
NTC = 5                      # residual-corrected taps (taps 0..4)
KIN = C * NTAP               # 2304 = per-filter fan-in
WTF = NTAP * NCO * NCI * P   # 4608 = packed lhsT free size


def _wt_off(t: int, co: int) -> int:
    # packed stationary layout: [co-chunk][tap][k-tile][oo] — co-major so
    # the first co block's ramp only waits on half the weight bytes
    return (co * NTAP + t) * NCI * P


def build(bp: int = BP):
    """Build + compile the per-core program for `bp` images per core."""
    nc = bacc.Bacc(
        "TRN2",
        target_bir_lowering=False,
        debug=False,
        enable_asserts=False,
        num_devices=N_CORES,
        enable_partition_id=False,
    )
    x8_d = nc.dram_tensor("x8", [bp, C, H * W], F8, kind="ExternalInput")
    r8_d = nc.dram_tensor("r8", [bp, C, H * W], F8, kind="ExternalInput")
    wb_d = nc.dram_tensor("wb", [O, KIN], BF16, kind="ExternalInput")
    # wp8[p, t, co, i, oo] = sign(w[co*128+oo, i*128+p, t]) in fp8 — the
    # packed DoubleRow stationary, host-marshalled.
    wp8_d = nc.dram_tensor("wp8", [P, WTF], F8, kind="ExternalInput")
    out_d = nc.dram_tensor("out", [bp, O, H, W], F32, kind="ExternalOutput")

    x8 = x8_d.ap().rearrange("n (c p) v -> p n c v", p=P)
    r8 = r8_d.ap().rearrange("n (c p) v -> p n c v", p=P)
    out = out_d.ap().rearrange("n c h w -> n c (h w)")

    COPY = mybir.ActivationFunctionType.Copy

    with tile.TileContext(nc) as tc:
        with (
            tc.tile_pool(name="const", bufs=1) as const_pool,
            tc.tile_pool(name="wstage", bufs=2) as wstage_pool,
            tc.tile_pool(name="xsf", bufs=4) as xsf_pool,
            tc.tile_pool(name="xpad", bufs=4) as xpad_pool,
            tc.tile_pool(name="otile", bufs=8) as out_pool,
            tc.tile_pool(name="psum", bufs=7, space="PSUM") as psum_pool,
            tc.tile_pool(name="warmps", bufs=1, space="PSUM") as warmps_pool,
        ):
            # ---- ramp-critical x8 chunk-1 DMAs: very first gpsimd ops ----
            # (the gpsimd ring can issue DMAs; enqueueing before its
            # memset backlog keeps the critical transfer start at ~engine
            # init instead of ~3.5us later). Image 0's staging is split
            # into 14-row pieces so the first matmul waits on 0.2MB, not
            # 0.8MB — the first-block group schedule below consumes
            # pieces as they land.
            HH = H // 2
            NPC = 4                  # image-0 row pieces
            PCR = H // NPC           # 14 rows per piece
            x8s0 = [[xsf_pool.tile([P, PCR * W], F8, name=f"x8s{k}_{ci}",
                                   bufs=1) for ci in range(NCI)]
                    for k in range(NPC)]
            # gpsimd's engine-driven transfers run at full rate even
            # before the clock boost — it carries ALL image-0 pieces in
            # consumption order (~5.3us chain, done before group 2 needs
            # them); the HWDGE rings are too slow pre-boost and would
            # starve the middle groups
            for k in range(NPC):
                for ci in range(NCI):
                    nc.gpsimd.dma_start(
                        x8s0[k][ci][:],
                        x8[:, 0, ci, k * PCR * W:(k + 1) * PCR * W])

            # ---- PE warmup: keep HAM at 2.4GHz while inputs stream in ----
            # warm operands memset on DVE: the gpsimd queue is busy with
            # the staging transfers above and would delay the warmups
            warm_l = const_pool.tile([P, P], BF16)
            warm_r = const_pool.tile([P, 512], BF16)
            nc.vector.memset(warm_l[:], 0.0)
            nc.vector.memset(warm_r[:], 0.0)
            zbias = const_pool.tile([P, 1], F32)
            zscr = const_pool.tile([P, 1], F32)
            nc.vector.memset(zbias[:], 0.0)
            warm_ps = warmps_pool.tile([P, 512], F32)
            # enough back-to-back zero matmuls to keep the PE busy from
            # engine-start until the first real matmul: the HAM 2.4GHz
            # boost engages after ~4us of SUSTAINED Tensor activity, and
            # an idle gap before the first real matmul re-arms the timer.
            # The ramp-critical ~1.4MB can't land before ~12us (early
            # DMA runs pre-boost too), so the warmups bridge that window
            # and real matmuls start already at full clock.
            N_WARM = 16  # covers data-ready jitter of ~12.5-16us
            for _ in range(N_WARM):
                nc.tensor.matmul(warm_ps[:], warm_l[:], warm_r[:],
                                 start=True, stop=True)
            # preload the Copy LUT on ACT before evictions need it
            nc.scalar.copy(zscr[:], zbias[:])

            wt8 = const_pool.tile([P, WTF], F8)
            a_all = const_pool.tile([P, NCO], F32)

            def pad_alloc():
                xp = xpad_pool.tile([P, NCI, PH, PW], F8, name="xp")
                nc.gpsimd.memset(xp[:, :, 0, :], 0.0)
                nc.gpsimd.memset(xp[:, :, PH - 1, :], 0.0)
                nc.gpsimd.memset(xp[:, :, 1:PH - 1, 0], 0.0)
                nc.gpsimd.memset(xp[:, :, 1:PH - 1, PW - 1], 0.0)
                return xp

            # ---- ramp-critical input DMAs --------------------------------
            # Only x8s0 + the first weight taps transfer up front; r8s0/wb
            # are enqueued on the scalar ring BEHIND an ACT copy that
            # blocks on x8s0's arrival (engine rings execute in order, so
            # the transfers physically start after the critical one).
            # The image-0 staging is split top/bottom so the interior
            # copies pipeline with the second half of the DMA.
            xp0 = pad_alloc()
            rp0 = pad_alloc()
            r8s0 = [xsf_pool.tile([P, NCI, HH * W], F8, name=f"r8s{h}",
                                  bufs=1) for h in range(2)]
            # chunk-0 pieces on the sync ring (chunk-1 rode gpsimd above);
            # the first weight taps share the window, the rest follows
            WCO = WTF // 2
            nc.scalar.dma_start(wt8[:, :WCO], wp8_d.ap()[:, :WCO])
            nc.sync.dma_start(wt8[:, WCO:], wp8_d.ap()[:, WCO:])
            wstages = [wstage_pool.tile([P, KIN], BF16, name="ws")
                       for _ in range(NCO)]
            wb = wb_d.ap()

            # ---- first-image interior copies: piece-pipelined DVE / ACT --
            for k in range(NPC):
                nc.vector.tensor_copy(
                    xp0[:, 0, 1 + k * PCR:1 + (k + 1) * PCR, 1:PW - 1],
                    x8s0[k][0][:].rearrange("p (h v) -> p h v", v=W))
                nc.scalar.copy(
                    xp0[:, 1, 1 + k * PCR:1 + (k + 1) * PCR, 1:PW - 1],
                    x8s0[k][1][:].rearrange("p (h v) -> p h v", v=W))
            # r8s0/wb start here — after all of x8s0+wt8 has landed, and
            # well before the corrective taps need them (~10us later)
            nc.scalar.dma_start(r8s0[0][:], r8[:, 0, :, :HH * W])
            nc.scalar.dma_start(r8s0[1][:], r8[:, 0, :, HH * W:])
            nc.scalar.dma_start(wstages[0][:], wb[0:P, :])
            nc.scalar.dma_start(wstages[1][:], wb[P:2 * P, :])
            for half in range(2):
                nc.vector.tensor_copy(
                    rp0[:, 0, 1 + half * HH:1 + (half + 1) * HH, 1:PW - 1],
                    r8s0[half][:, 0].rearrange("p (h v) -> p h v", v=W))
                nc.scalar.copy(
                    rp0[:, 1, 1 + half * HH:1 + (half + 1) * HH, 1:PW - 1],
                    r8s0[half][:, 1].rearrange("p (h v) -> p h v", v=W))

            # |w| means (DVE) — per co chunk, before that chunk's evictions
            def emit_reduce(c2):
                asum = wstage_pool.tile([P, 1], F32, name="asum", bufs=2)
                nc.vector.tensor_reduce(
                    asum[:], wstages[c2][:], axis=mybir.AxisListType.X,
                    op=mybir.AluOpType.add, apply_absolute_value=True,
                )
                nc.vector.tensor_scalar_mul(
                    a_all[:, c2:c2 + 1], asum[:], 1.0 / KIN
                )

            def lhs(t, co):
                off = _wt_off(t, co)
                return wt8[:, off:off + NCI * P].rearrange(
                    "p (i o) -> p i o", i=NCI)

            def rhs(xp, t, ht):
                kh, kw = divmod(t, KW)
                r0 = ht * HT + kh
                return xp[:, :, r0:r0 + HT, kw:kw + W]

            def evict(ps, n, co, ht, split=False):
                ot = out_pool.tile([P, NFREE], F32, name="ot")
                if ht % 2 == 0:
                    nc.vector.tensor_scalar_mul(
                        ot[:], ps[:], a_all[:, co:co + 1])
                else:
                    nc.scalar.activation(
                        ot[:], ps[:], COPY, bias=0.0,
                        scale=a_all[:, co:co + 1])
                dst = out[n, co * P:(co + 1) * P,
                          ht * NFREE:(ht + 1) * NFREE]
                if split:
                    # final-block stores alternate between the idle
                    # gpsimd engine (DIRECT2D, ~1.5us/tile) and the
                    # HWDGE queues so the two paths drain in parallel
                    if ht % 2 == 0:
                        nc.gpsimd.dma_start(dst, ot[:])
                    else:
                        nc.scalar.dma_start(dst[0:P // 2], ot[0:P // 2, :])
                        nc.scalar.dma_start(dst[P // 2:P], ot[P // 2:P, :])
                else:
                    nc.scalar.dma_start(dst, ot[:])

            DR = mybir.MatmulPerfMode.DoubleRow

            # ---- main conv loop ------------------------------------------
            xp, rp = xp0, rp0
            for n in range(bp):
                x8s = r8s = None
                for co in range(NCO):
                    if n == 0:
                        emit_reduce(co)
                    final = (n == bp - 1 and co == NCO - 1)
                    pss = [psum_pool.tile([P, NFREE], F32, name="ps")
                           for _ in range(NHT)]
                    if not final:
                        # the very first block consumes staging pieces as
                        # they land: each group's row blocks fit within
                        # the pieces available at that point, keeping the
                        # PE fed without a clock-dropping idle gap
                        if n == 0 and co == 0:
                            ht_groups = [[0], [1, 2], [3, 4], [5, 6]]
                        else:
                            ht_groups = [range(NHT)]
                        for hts in ht_groups:
                            for t in range(NTAP):
                                lt = lhs(t, co)
                                for ht in hts:
                                    nc.tensor.matmul(
                                        pss[ht][:], lt, rhs(xp, t, ht),
                                        start=(t == 0), stop=False,
                                        perf_mode=DR)
                        for hts in ht_groups:
                            for t in range(NTC):
                                lt = lhs(t, co)
                                for ht in hts:
                                    nc.tensor.matmul(
                                        pss[ht][:], lt, rhs(rp, t, ht),
                                        start=False, stop=(t == NTC - 1),
                                        perf_mode=DR)
                                    if t == NTC - 1:
                                        evict(pss[ht], n, co, ht)
                    else:
                        # rowblock-outer: pipeline evictions with matmuls
                        for ht in range(NHT):
                            for t in range(NTAP):
                                nc.tensor.matmul(
                                    pss[ht][:], lhs(t, co), rhs(xp, t, ht),
                                    start=(t == 0), stop=False,
                                    perf_mode=DR)
                            for t in range(NTC):
                                nc.tensor.matmul(
                                    pss[ht][:], lhs(t, co), rhs(rp, t, ht),
                                    start=False, stop=(t == NTC - 1),
                                    perf_mode=DR)
                            evict(pss[ht], n, co, ht, split=True)
                    # next image's prefetch DMAs ride the scalar ring
                    # behind co-0's evictions: the transfers start only
                    # once the ramp/previous traffic has drained, without
                    # stealing HBM bandwidth from the critical loads (the
                    # sync engine would run ahead and start them at t=0).
                    # The interior copies are emitted between co0 and co1
                    # so DVE/ACT reach them during co1's compute, a full
                    # co-block before the next image's matmuls need them.
                    if co == 0 and n + 1 < bp:
                        x8s = xsf_pool.tile([P, NCI, H * W], F8,
                                            name="x8f", bufs=2)
                        r8s = xsf_pool.tile([P, NCI, H * W], F8,
                                            name="r8f", bufs=2)
                        nc.scalar.dma_start(x8s[:], x8[:, n + 1])
                        nc.scalar.dma_start(r8s[:], r8[:, n + 1])
                        nxp = pad_alloc()
                        nrp = pad_alloc()
                        nc.vector.tensor_copy(
                            nxp[:, 0, 1:PH - 1, 1:PW - 1],
                            x8s[:, 0].rearrange("p (h v) -> p h v", v=W))
                        nc.scalar.copy(
                            nxp[:, 1, 1:PH - 1, 1:PW - 1],
                            x8s[:, 1].rearrange("p (h v) -> p h v", v=W))
                        nc.vector.tensor_copy(
                            nrp[:, 0, 1:PH - 1, 1:PW - 1],
                            r8s[:, 0].rearrange("p (h v) -> p h v", v=W))
                        nc.scalar.copy(
                            nrp[:, 1, 1:PH - 1, 1:PW - 1],
                            r8s[:, 1].rearrange("p (h v) -> p h v", v=W))
                if n + 1 < bp:
                    xp, rp = nxp, nrp

    nc.compile()
    return nc


_NC_CACHE: dict[int, object] = {}


def _get_nc(bp: int = BP):
    if bp not in _NC_CACHE:
        _NC_CACHE[bp] = build(bp)
    return _NC_CACHE[bp]


def make_in_maps(x: np.ndarray, weight: np.ndarray, n_cores: int = N_CORES,
                 bp: int = BP):
    x = np.ascontiguousarray(x, dtype=np.float32).reshape(B, C, H * W)
    weight = np.ascontiguousarray(weight, dtype=np.float32)
    x8 = x.astype(ml_dtypes.float8_e4m3)
    r8 = (x - x8.astype(np.float32)).astype(ml_dtypes.float8_e4m3)
    # wp8[p, t, co, i, oo] = sign(w[co*128+oo, i*128+p, t]) — exact in fp8
    wv = np.sign(weight).reshape(NCO, P, NCI, P, NTAP)  # [co, oo, i, p, t]
    wp8 = np.ascontiguousarray(
        wv.transpose(3, 0, 4, 2, 1)                     # [p, co, t, i, oo]
    ).reshape(P, WTF).astype(ml_dtypes.float8_e4m3)
    wb = weight.reshape(O, KIN).astype(ml_dtypes.bfloat16)
    return [
        {"x8": x8[i * bp:(i + 1) * bp], "r8": r8[i * bp:(i + 1) * bp],
         "wb": wb, "wp8": wp8}
        for i in range(n_cores)
    ]


def kernel(x: np.ndarray, weight: np.ndarray) -> np.ndarray:
    nc = _get_nc(BP)
    in_maps = make_in_maps(x, weight)
    res = run_bass_kernel_spmd(nc, in_maps, core_ids=list(range(N_CORES)))
    out = np.empty((B, O, H, W), dtype=np.float32)
    for i in range(N_CORES):
        out[i * BP:(i + 1) * BP] = res.results[i]["out"].reshape(BP, O, H, W)
    return out
